# revision 15
# baseline (speedup 1.0000x reference)
"""Slot-attention kernel for Trainium2, SPMD over 8 NeuronCores.

Reference computation (per batch element b):
  query[b,n,:] = q[n,b,:] @ qw[n]          (n = 32 query slots)
  keyp [b,m,:] = k[m,b,:] @ kw[m]          (m = 32 key slots)
  value[b,m,:] = k[m,b,:] @ vw[m]
  logits[b,n,m] = query[b,n,:]·keyp[b,m,:] / 16
  attn = softmax_m(logits)
  out[n,b,:] = sum_m attn[b,n,m] * value[b,m,:]

Sharding: data-parallel over batch (4096 -> 512 per core), weights replicated.
Host pre-casts to bf16 and pre-transposes q/k to [slot, dim, batch] so every
DMA is contiguous and the contraction dim (dim) lands on SBUF partitions.

Per-core schedule (two batch halves of 256; phases B/C per 128-batch
sub-half):
  A) per-slot projections on PE with N=256 moving (full half); K/V weights
     loaded once and kept resident, Q weights streamed per half. Full-bank
     [128,512] psum tiles, one contiguous psum->sbuf copy per slot.
     V lands as [b, sh, m, o] quarter slabs; each completed quarter is
     shuffled into per-sub-half V32Q[32r+m, g, o] via r-quad DMAs on the
     idle GpSimd SWDGE (4 strided partitions per DMA, 32 per sub-half).
  B) logits via col-tiled matmuls (batch = 32j + g within the sub-half),
     16 batches per full psum bank, one exp per bank; rowsums + reciprocals
     feed the output scaling.
  C) DVE 32x32 transposes pack attn^T; attn@value as 4-way diagonal
     tile-packed matmuls, two groups per psum bank; psum->sbuf copies fold
     the softmax normalization; bf16 output DMA on the scalar HWDGE ring
     (host casts back to f32).
"""

import numpy as np
import ml_dtypes

import concourse.bass as bass
from concourse import bacc
import concourse.mybir as mybir
import concourse.tile as tile
from concourse.bass_utils import run_bass_kernel_spmd
# NOTE: --enable-ldw-opt=true was tried and rejected by walrus codegen:
# "InstLdweights is not compatible with LDW optimization" for the
# tile_position ldweights used in the logits/AV phases.

BF16 = mybir.dt.bfloat16
F32 = mybir.dt.float32

NQ = 32          # query slots
NK = 32          # key slots
D = 256          # input dim (contraction of projections)
A = 256          # attn dim (contraction of logits)
O = 256          # out dim
BS = 4096
N_CORES = 8
BS_CORE = BS // N_CORES   # 512


def build_kernel(bs_core=BS_CORE, n_halves=2):
    """Builds the per-core Bass graph. bs_core must be divisible by 256."""
    nc = bacc.Bacc()

    b_h = bs_core // n_halves          # batch per half (256)
    b_s = b_h // 2                     # batch per sub-half (128)
    n_groups = b_s // 4                # groups per sub-half (32); b = 32j + g

    qT = nc.declare_dram_parameter("qT", [NQ, D, bs_core], BF16, isOutput=False)
    kT = nc.declare_dram_parameter("kT", [NK, D, bs_core], BF16, isOutput=False)
    qwD = nc.declare_dram_parameter("qw", [NQ, D, A], BF16, isOutput=False)
    # K and V weights merged: [slot, d, 2 (k/v), a]
    kvwD = nc.declare_dram_parameter("kvw", [NK, D, 2, A], BF16,
                                     isOutput=False)
    out = nc.declare_dram_parameter("out", [NQ, bs_core, O], BF16,
                                    isOutput=True)

    SG = 2  # slots per input DMA group
    # [slot, d, b] -> partition = d%128, chunk c = d//128
    qT_g = qT.rearrange("(sg s) (c p) b -> sg p (s c) b", p=128, s=SG)
    kT_g = kT.rearrange("(sg s) (c p) b -> sg p (s c) b", p=128, s=SG)
    qw_g = qwD.rearrange("(sg s) (c p) a -> sg p (s c) a", p=128, s=SG)
    kvw_g = kvwD.rearrange("(sg s) (c p) w a -> sg p (s c) (w a)", p=128, s=SG)

    n_sg = NQ // SG

    with tile.TileContext(nc) as tc:
        with (
            tc.tile_pool(name="const", bufs=1) as const_pool,
            tc.tile_pool(name="win", bufs=2) as win,
            tc.tile_pool(name="xin", bufs=3) as xin,
            tc.tile_pool(name="big", bufs=1) as big,
            tc.tile_pool(name="vnp", bufs=2) as vnp,
            tc.tile_pool(name="vqp", bufs=2) as vqp,
            tc.tile_pool(name="outp", bufs=2) as outp,
            tc.tile_pool(name="smp", bufs=4) as smp,
            tc.tile_pool(name="etp", bufs=6) as etp,
            tc.tile_pool(name="proj_ps", bufs=4, space="PSUM") as proj_ps,
            tc.tile_pool(name="lg_ps", bufs=2, space="PSUM") as lg_ps,
            tc.tile_pool(name="av_ps", bufs=2, space="PSUM") as av_ps,
        ):
            # resident K/V weights: [a-part, slot, c, (kw|vw), a]
            KVW = const_pool.tile([128, NK, 2, 2, A], BF16, tag="KVW")

            # ~4.5us of dummy back-to-back matmuls (one weight load, many
            # short MMs) while the first input DMAs are in flight, so the
            # PE_HAM clock gate reaches K=8/8 before the real work starts
            warm = const_pool.tile([128, 128], BF16, tag="warm")
            nc.vector.memset(warm[:, :], 0.0)
            wps = av_ps.tile([128, 512], F32, tag="av")
            for _ in range(28):
                nc.tensor.matmul(wps[:, 0:128], lhsT=warm, rhs=warm,
                                 start=True, stop=True)

            for half in range(n_halves):
                b0 = half * b_h
                # ---- Phase A: projections ----
                QTs = big.tile([128, NQ, 2, b_h], BF16, tag="QTs")
                KTs = big.tile([128, NK, 2, b_h], BF16, tag="KTs")
                # V32Q[32r+m, g, o] = value[b0 + sh*128 + 32r + g][m, o]
                V32Q = [vqp.tile([128, n_groups, O], BF16, tag="V32Q",
                                 name=f"V32Q_{half}_{shh}")
                        for shh in range(2)]

                VNq = None
                for sg in range(n_sg):
                    qts = xin.tile([128, SG, 2, b_h], BF16, tag="qts")
                    nc.sync.dma_start(out=qts,
                                      in_=qT_g[sg, :, :, b0:b0 + b_h])
                    kts = xin.tile([128, SG, 2, b_h], BF16, tag="kts")
                    nc.sync.dma_start(out=kts,
                                      in_=kT_g[sg, :, :, b0:b0 + b_h])
                    # weight loads ride the scalar HWDGE ring so the sync
                    # ring only carries the q/k data stream
                    wsg = win.tile([128, SG, 2, A], BF16, tag="wsg")
                    nc.scalar.dma_start(out=wsg, in_=qw_g[sg])
                    if half == 0:
                        s0 = sg * SG
                        nc.scalar.dma_start(
                            out=KVW[:, s0:s0 + SG, :, :, :],
                            in_=kvw_g[sg].rearrange(
                                "p (s c) wa -> p s c wa", s=SG),
                        )

                    if sg % 4 == 0:
                        # value quarter slab [b%128, sub-half, mi, o]
                        VNq = vnp.tile([128, 2, 8, O], BF16, tag="VNq")

                    for si in range(SG):
                        s = sg * SG + si
                        # Q projection: psum [a-tile, 256] full bank
                        ps = proj_ps.tile([128, 2, b_h], F32, tag="ps")
                        for t in range(2):
                            for c in range(2):
                                nc.tensor.matmul(
                                    ps[:, t, :],
                                    lhsT=wsg[:, si, c, t * 128:(t + 1) * 128],
                                    rhs=qts[:, si, c, :],
                                    start=(c == 0),
                                    stop=(c == 1),
                                )
                        nc.scalar.mul(QTs[:, s, :, :], ps, 1.0 / 16.0)
                        # K projection
                        ps = proj_ps.tile([128, 2, b_h], F32, tag="ps")
                        for t in range(2):
                            for c in range(2):
                                nc.tensor.matmul(
                                    ps[:, t, :],
                                    lhsT=KVW[:, s, c, 0,
                                             t * 128:(t + 1) * 128],
                                    rhs=kts[:, si, c, :],
                                    start=(c == 0),
                                    stop=(c == 1),
                                )
                        nc.vector.tensor_copy(out=KTs[:, s, :, :], in_=ps)
                    # V projection: stationary = k batch-chunk, moving = vw
                    # -> psum [b_chunk, si, o]; slot-pair per bank
                    for sh in range(2):
                        ps = proj_ps.tile([128, SG, O], F32, tag="ps")
                        for si in range(SG):
                            s = sg * SG + si
                            for c in range(2):
                                nc.tensor.matmul(
                                    ps[:, si, :],
                                    lhsT=kts[:, si, c,
                                             sh * 128:(sh + 1) * 128],
                                    rhs=KVW[:, s, c, 1, :],
                                    start=(c == 0),
                                    stop=(c == 1),
                                )
                        mrow = (sg * SG) % 8
                        if sh == 0:
                            nc.scalar.copy(
                                out=VNq[:, sh, mrow:mrow + SG, :], in_=ps)
                        else:
                            nc.vector.tensor_copy(
                                out=VNq[:, sh, mrow:mrow + SG, :], in_=ps)

                    if sg % 4 == 3:
                        # shuffle the completed quarter (slots 8qq..8qq+8)
                        # into the per-sub-half V32Q: row 32r + 8qq + mi
                        # <- batches pb = 32r + g of sub-half sh (one DMA
                        # per (sh, mi): 4 strided partitions, 128 KB)
                        qq = sg // 4
                        for sh in range(2):
                            for mi in range(8):
                                base = 8 * qq + mi
                                nc.gpsimd.dma_start(
                                    out=V32Q[sh][base:base + 97:32, :, :],
                                    in_=VNq[:, sh, mi, :],
                                )

                # ---- Phase B (both sub-halves back to back, so the PE
                # stream stays dense while softmax handoffs run) ----
                Es, rss = [], []
                for sh in range(2):
                    rs = big.tile([128, n_groups], F32, tag=f"rs{sh}",
                                  name=f"rs_{half}_{sh}")
                    E = big.tile([128, n_groups, NK], BF16, tag=f"E{sh}",
                                 name=f"E_{half}_{sh}")
                    Es.append(E)
                    rss.append(rs)

                    for gb in range(n_groups // 16):
                        lg = lg_ps.tile([128, 16, NK], F32, tag="lg")
                        for qi in range(16):
                            g = 16 * gb + qi
                            bl = sh * b_s + g  # batch col in QTs/KTs, j adds 32
                            for c in range(2):
                                for j in range(4):
                                    nc.tensor.matmul(
                                        lg[32 * j:32 * (j + 1), qi, :],
                                        lhsT=QTs[:, :, c, bl + 32 * j],
                                        rhs=KTs[:, :, c, bl + 32 * j],
                                        start=(c == 0),
                                        stop=(c == 1),
                                        tile_position=(0, 32 * j),
                                        skip_group_check=True,
                                    )
                        # softmax over m without max-subtraction: logits
                        # carry the 1/16 so |logit| <= ~2 and exp cannot
                        # overflow; normalization folds into the output copy
                        nc.scalar.activation(
                            out=E[:, 16 * gb:16 * gb + 16, :].rearrange(
                                "p a b -> p (a b)"),
                            in_=lg.rearrange("p a b -> p (a b)"),
                            func=mybir.ActivationFunctionType.Exp,
                        )
                        sm = smp.tile([128, 16], F32, tag="sm")
                        nc.vector.reduce_sum(
                            out=sm, in_=E[:, 16 * gb:16 * gb + 16, :],
                            axis=mybir.AxisListType.X,
                        )
                        nc.vector.reciprocal(out=rs[:, 16 * gb:16 * gb + 16],
                                             in_=sm)
                        # normalize E in place (attn = exp * 1/rowsum) so
                        # phase C copies need no per-group scaling
                        nc.vector.tensor_mul(
                            out=E[:, 16 * gb:16 * gb + 16, :],
                            in0=E[:, 16 * gb:16 * gb + 16, :],
                            in1=rs[:, 16 * gb:16 * gb + 16].unsqueeze(
                                2).to_broadcast([128, 16, NK]),
                        )

                # ---- Phase C: attn @ value ----
                for sh in range(2):
                    b0s = b0 + sh * b_s
                    E = Es[sh]
                    g_chunk = 8
                    for g0 in range(0, n_groups, g_chunk):
                        OUTo = outp.tile([128, g_chunk, O], BF16, tag="OUTo")
                        # one DVE pass transposes all 8 groups' 32x32
                        # attn blocks: te8[32j+m, 32gi+n] = E[32j+n, g0+gi, m]
                        te8 = etp.tile([128, g_chunk, NK], BF16, tag="te8")
                        nc.vector.transpose(
                            out=te8.rearrange("p a b -> p (a b)"),
                            in_=E[:, g0:g0 + g_chunk, :].rearrange(
                                "p a b -> p (a b)"))
                        for gp in range(g_chunk // 2):
                            g = g0 + 2 * gp
                            av = av_ps.tile([128, 2, O], F32, tag="av")
                            for gg in range(2):
                                for j in range(4):
                                    nc.tensor.matmul(
                                        av[32 * j:32 * (j + 1), gg, :],
                                        lhsT=te8[32 * j:32 * (j + 1),
                                                 2 * gp + gg, :],
                                        rhs=V32Q[sh][32 * j:32 * (j + 1),
                                                     g + gg, :],
                                        start=True, stop=True,
                                        tile_position=(32 * j, 32 * j),
                                        skip_group_check=True,
                                    )
                            # plain full-bank psum -> sbuf copy
                            if gp % 2 == 0:
                                nc.scalar.copy(
                                    out=OUTo[:, 2 * gp:2 * gp + 2, :],
                                    in_=av)
                            else:
                                nc.vector.tensor_copy(
                                    out=OUTo[:, 2 * gp:2 * gp + 2, :],
                                    in_=av)
                        # flush on the scalar HWDGE ring (sync ring carries
                        # the input loads)
                        for j in range(4):
                            nc.scalar.dma_start(
                                out=out[:, b0s + 32 * j + g0:
                                        b0s + 32 * j + g0 + g_chunk, :],
                                in_=OUTo[32 * j:32 * (j + 1), :, :],
                            )
    return nc


def _prep_inputs(q, k, query_weight, key_weight, value_weight, bs_core):
    bf = ml_dtypes.bfloat16
    qw = np.ascontiguousarray(query_weight).astype(bf)
    kvw = np.ascontiguousarray(
        np.stack((key_weight, value_weight), axis=2)).astype(bf)
    in_maps = []
    for i in range(N_CORES):
        sl = slice(i * bs_core, (i + 1) * bs_core)
        qTb = np.ascontiguousarray(q[:, sl, :].transpose(0, 2, 1)).astype(bf)
        kTb = np.ascontiguousarray(k[:, sl, :].transpose(0, 2, 1)).astype(bf)
        in_maps.append({"qT": qTb, "kT": kTb, "qw": qw, "kvw": kvw})
    return in_maps


_NC_CACHE = {}


def _get_nc(bs_core, n_halves=2):
    key = (bs_core, n_halves)
    if key not in _NC_CACHE:
        nc = build_kernel(bs_core, n_halves)
        nc.finalize()
        _NC_CACHE[key] = nc
    return _NC_CACHE[key]


def kernel(q, k, query_weight, key_weight, value_weight, _trace=False):
    nc = _get_nc(BS_CORE)
    in_maps = _prep_inputs(q, k, query_weight, key_weight, value_weight, BS_CORE)
    res = run_bass_kernel_spmd(nc, in_maps, core_ids=list(range(N_CORES)),
                               trace=_trace)
    outs = [res.results[i]["out"] for i in range(N_CORES)]
    full = np.concatenate(outs, axis=1).astype(np.float32)
    if _trace:
        return full, res
    return full
